# revision 21
# baseline (speedup 1.0000x reference)
"""DeepTensorNeuralNetwork (DTNN / gnn_message_passing) Trainium2 kernel.

Math (per reference):
    d_sum = distance.sum(axis=2)                                  # (B,N,R)
    for l in 0..2:
        cf = x @ Wcf[l].T + bcf[l]                                # (B,N,H)
        df = d_sum @ Wdf[l].T + N*bdf[l]                          # (B,N,H)
        h  = (cf*df) @ Wfc[l].T                                   # (B,N,F)
        x  = h + tanh(h)
    g = x.sum(axis=1); out = (g @ fc0.T + b0) @ ow.T + ob         # (B,1)

Strategy: data-parallel over batch across 8 NeuronCores (8 batches/core).
`distance` is cast to fp16 on the host (like the baseline already did for
x and all weights; quantization error of the j-sum is ~1e-4 relative vs
the 2e-2 gate), which halves the dominant HBM stream to 2.1 MB/batch
(~5.0 us/batch at the measured ~420 GB/s; 17.8 MB total -> ~44 us).
Every large DMA is split across the sync+scalar HWDGE rings to keep the
two rings byte-balanced (an unbalanced ring delays that ring's batches).

The j-reduction is a DVE binary fold tree, all levels fp16 in the 2x
mode (~0.52 ns/output/lane + ~150 ns/op overhead, so FEWER/WIDER ops
win: full 8192->128 trees, 6 ops/batch).  The tree stops at 128 cols =
(j2, r): the last 2:1 j-fold is absorbed into the df matmul by doubling
the Wdf lhsT rows (contract 128 instead of 64; free).  The last batch
uses two half-trees so the final dsum half only gates on the last DMA
chunk.  Measured limits: DVE total work (~48 us: folds 40 + muls 4 +
tail 4) slightly exceeds the stream, so dsum(b7) lands ~10 us after
stream-end; GpSimd tensor ops run ~3.8 ns/elem (no use; they do NOT
serialize against DVE, contrary to the old note, just slow); ACT is the
co-bottleneck (~34 us: 4 bias copies + copy/tanh per layer-group at
NG-proportional cost, exec-queue depth 0).

The 3-layer pipeline runs per batch-group (4,2,2).  For the throughput
groups G0/G1: PE matmuls -> ACT bias copies (fp16 SBUF) -> one DVE mul
per H-half (2x mode), residual handled as cf_{l+1} = mm(Wcf,h_sb) +
mm(Wcf,th) accumulated in PSUM (no DVE adds).  The final group [6,7]
is latency-optimized (it runs after the stream ends): df for l>=1 is
precomputed into SBUF off the critical path, cf accumulates in PSUM,
and the DVE does m = (cf_psum + bias) * df_sbuf in ONE fused
scalar_tensor_tensor; the residual is tanh -> one DVE add, and the
head is a single matmul on x3.  PSUM: bank sets A (cf,df,h) for G0,
B for G1, A+B c-split for the final group, + shared tr/hd = 8 banks.

Known fixed costs: ~6 us preamble (excluded from exec time), ~9.5 us
teardown (288 per-event semaphore resets, framework-fixed).  The core
DVFS state adds run-to-run variance: warm consecutive runs measure
~82-84 us; cold or power-capped runs run all engines at 5/6 clock and
measure ~90-100 us.
"""

import numpy as np

B, N, F, R, H = 64, 128, 128, 64, 256
L = 3
NCORES = 8
BL = B // NCORES   # batches per core
GROUPS = ((0, 1, 2, 3), (4, 5), (6, 7))
GP_SET = ()        # GpSimd fold assist: measured ~3.8 ns/elem (10x slower
                   # than DVE 2x) -- gating any fold level on it loses time

# wpack layout, fp32 columns (bf-cols = fp16-element columns of the
# bitcast view at 2x the fp32 column index):
#   [0, 384)      wcf lhsT f16 : bf-col l*H+h           = Wcf_w[l, h, f]
#   [384, 768)    wfc lhsT f16 : bf-col (l*2+c)*F+f     = Wfc_w[l, f, c*128+hc]
#   [768, 774)    cf bias fp32 : col l*2+c              = Wcf_b[l, c*128+h]
#   [774, 780)    df bias fp32 : col l*2+c              = N * Wdf_b[l, c*128+h]
#   [780, 781)    head lhsT f16: bf-col 0               = (out_w @ fc0_w)[0, f]
#   [784, 1168)   wdf2 lhsT f16: bf-col l*H+h, row j2*64+r = Wdf_w[l, h, r]
#   [1168, 1680)  x f16        : bf-col b*N+n           = x[b_local, n, f]
#   [1680, 1744)  identity f16-packed
BCF_OFF = 768
BDF_OFF = 774
HEAD_OFF = 780
WDF_OFF = 784
XOFF = 1168
IDOFF = 1680
WCOLS = 1744

_CACHE = {}


def _build_program():
    import concourse.bass as bass
    from concourse import bacc
    import concourse.tile as tile
    from concourse import mybir
    from concourse import hw_specs

    # The Tile scheduler orders the (runtime in-order) engine queues from a
    # simulation that models HBM at ~332 GB/s; this device streams ~420.
    # Build with the measured rate so simulated data-arrival (and hence the
    # queue order) matches hardware; restored right after the build.
    _dma_cycle_prev = hw_specs.TRN2Spec.DMA_CYCLE
    hw_specs.TRN2Spec.DMA_CYCLE = 1e9 / (420e9 / 128)
    try:
        return _build_program_inner(bass, bacc, tile, mybir)
    finally:
        hw_specs.TRN2Spec.DMA_CYCLE = _dma_cycle_prev


def _build_program_inner(bass, bacc, tile, mybir):

    f32 = mybir.dt.float32
    f16 = mybir.dt.float16
    AX = mybir.AxisListType
    AF = mybir.ActivationFunctionType

    nc = bacc.Bacc("TRN2")
    dist = nc.declare_dram_parameter("dist", [BL, N, N * R], f16, isOutput=False)
    wpack = nc.declare_dram_parameter("wpack", [128, WCOLS], f32, isOutput=False)
    out_ext = nc.declare_dram_parameter("out", [BL, 1], f32, isOutput=True)

    with tile.TileContext(nc) as tc:
        with (
            tc.tile_pool(name="consts", bufs=1) as consts,
            tc.tile_pool(name="dist", bufs=4) as dist_pool,
            tc.tile_pool(name="fold", bufs=2) as fold_pool,
            tc.tile_pool(name="dsum", bufs=2) as dsum_pool,
            tc.tile_pool(name="work", bufs=2) as work,
            tc.tile_pool(name="psA", bufs=1, space="PSUM") as psA,
            tc.tile_pool(name="psB", bufs=1, space="PSUM") as psB,
            tc.tile_pool(name="psS", bufs=1, space="PSUM") as psS,
        ):
            # ---- DMA: distance stream on sync+scalar rings ------------
            # Stream order interleaves the LAST group's (b6, b7) halves so
            # their first halves fold during early DVE idle and only their
            # second halves arrive at stream end (dsum_b7 ~ stream-bound).
            dist_tiles = {}

            def start_dist_dma(b, hf=None):
                if b in dist_tiles:
                    t = dist_tiles[b]
                else:
                    tag = "dist67" if b >= 6 else "dist"
                    bufs = 2 if b >= 6 else 4
                    t = dist_pool.tile([N, N * R], f16, tag=tag,
                                       name="dist_t", bufs=bufs)
                    dist_tiles[b] = t
                dflat = dist[b, :, :]
                lo = 0 if hf in (None, 0) else 4096
                hi = 8192 if hf in (None, 1) else 4096
                mid = (lo + hi) // 2
                nc.sync.dma_start(out=t[:, lo:mid], in_=dflat[:, lo:mid])
                nc.scalar.dma_start(out=t[:, mid:hi], in_=dflat[:, mid:hi])

            start_dist_dma(0)
            start_dist_dma(1)

            wp = consts.tile([128, WCOLS], f32)
            hwc = WCOLS // 2
            nc.sync.dma_start(out=wp[:, 0:hwc], in_=wpack[:, 0:hwc])
            nc.scalar.dma_start(out=wp[:, hwc:WCOLS],
                                in_=wpack[:, hwc:WCOLS])
            wb = wp.bitcast(f16)  # (128, 2*WCOLS) f16 view
            ident = wb[:, 2 * IDOFF : 2 * IDOFF + 128]
            out_acc = consts.tile([1, BL], f32)

            start_dist_dma(2)

            def wcf_l(l, c):
                o = l * H + c * 128
                return wb[:, o : o + 128]

            def wdf_l(l, c):
                o = 2 * WDF_OFF + l * H + c * 128
                return wb[:, o : o + 128]

            def wfc_l(l, c):
                o = 2 * 384 + (l * 2 + c) * F
                return wb[:, o : o + F]

            def bcf_l(l, c):
                o = BCF_OFF + l * 2 + c
                return wp[:, o : o + 1]

            def bdf_l(l, c):
                o = BDF_OFF + l * 2 + c
                return wp[:, o : o + 1]

            head_w = wb[:, 2 * HEAD_OFF : 2 * HEAD_OFF + 1]

            def xcols(b0, b1):
                return wb[:, 2 * XOFF + b0 * N : 2 * XOFF + b1 * N]

            # ---- folds -------------------------------------------------
            dsums = {}

            def fold_full(b, gp=False, then_dma=()):
                """One tree 8192 -> 128 cols (j2, r)."""
                src = dist_tiles.pop(b)
                for args in then_dma:
                    start_dist_dma(*args)
                s = fold_pool.tile([N, 4096], f16, tag="s", name="s")
                if gp:
                    nc.gpsimd.tensor_add(s[:, 0:2048], src[:, 0:2048],
                                         src[:, 4096:6144])
                    nc.vector.tensor_add(s[:, 2048:4096], src[:, 2048:4096],
                                         src[:, 6144:8192])
                else:
                    nc.vector.tensor_add(s, src[:, 0:4096], src[:, 4096:8192])
                t = fold_pool.tile([N, 2048], f16, tag="t", name="t")
                dsum = dsum_pool.tile([N, 128], f16, tag="dsum", name="dsum", bufs=8)
                cur, other, w = s, t, 2048
                while w >= 128:
                    dst = dsum if w == 128 else other[:, 0:w]
                    nc.vector.tensor_add(dst, cur[:, 0:w], cur[:, w : 2 * w])
                    cur, other = other, cur
                    w //= 2
                dsums[b] = dsum

            def fold_half(b, hf, pop=False):
                """Half-tree (j-range hf) -> dsum[:, hf*64 : hf*64+64]."""
                src_t = dist_tiles[b]
                if b not in dsums:
                    dsums[b] = dsum_pool.tile([N, 128], f16, tag="dsum",
                                              name="dsumh", bufs=8)
                dsum = dsums[b]
                off = hf * 4096
                s = fold_pool.tile([N, 2048], f16, tag=f"hs{hf}", name="hs")
                nc.vector.tensor_add(s, src_t[:, off : off + 2048],
                                     src_t[:, off + 2048 : off + 4096])
                t = fold_pool.tile([N, 1024], f16, tag=f"ht{hf}", name="ht")
                cur, other, w = s, t, 1024
                while w >= 64:
                    dst = dsum[:, hf * 64 : hf * 64 + 64] if w == 64 \
                        else other[:, 0:w]
                    nc.vector.tensor_add(dst, cur[:, 0:w], cur[:, w : 2 * w])
                    cur, other = other, cur
                    w //= 2
                if pop:
                    dist_tiles.pop(b)

            # ---- group state / layer pipeline --------------------------
            gstate = {}

            def ps_pool(gi, c=0):
                """G0 -> A, G1 -> B, final group -> A (c=0) / B (c=1)."""
                if gi == 0:
                    return psA
                if gi == 1:
                    return psB
                return psA if c == 0 else psB

            def emit_trs(gi):
                bs = GROUPS[gi]
                NG = len(bs) * N
                dsT = dsum_pool.tile([128, 4 * N], f16, tag="dsT",
                                     name=f"dsT{gi}")
                for k, b in enumerate(bs):
                    trp = psS.tile([128, N], f16, tag="tr", name="trp")
                    nc.tensor.transpose(trp, dsums.pop(b), ident)
                    nc.scalar.activation(
                        out=dsT[:, k * N : (k + 1) * N], in_=trp, func=AF.Copy
                    )
                gstate[gi] = {"dsT": dsT[:, 0:NG], "NG": NG, "bs": bs,
                              "xc": xcols(bs[0], bs[-1] + 1)}

            def emit_layer(gi, l):
                """Throughput path (G0/G1): ACT bias copies, f16 SBUF muls.
                c halves share one cf bank and one df bank serially."""
                st = gstate[gi]
                NG = st["NG"]
                ms = []
                for c in range(2):
                    cfp = ps_pool(gi, 0).tile([128, 4 * N], f32, tag="cf",
                                              name="cfp")[:, 0:NG]
                    if l == 0:
                        nc.tensor.matmul(cfp, wcf_l(l, c), st["xc"],
                                         start=True, stop=True)
                    else:
                        nc.tensor.matmul(cfp, wcf_l(l, c), st["hsb"],
                                         start=True, stop=False)
                        nc.tensor.matmul(cfp, wcf_l(l, c), st["th"],
                                         start=False, stop=True)
                    cfs = work.tile([128, 4 * N], f16, tag=f"cfs{gi % 2}{c}",
                                    name="cfs")[:, 0:NG]
                    nc.scalar.activation(out=cfs, in_=cfp, func=AF.Identity,
                                         bias=bcf_l(l, c))
                    dfp = ps_pool(gi, 1).tile([128, 4 * N], f32, tag="df",
                                              name="dfp")[:, 0:NG]
                    nc.tensor.matmul(dfp, wdf_l(l, c), st["dsT"],
                                     start=True, stop=True)
                    dfs = work.tile([128, 4 * N], f16, tag=f"dfs{gi % 2}{c}",
                                    name="dfs")[:, 0:NG]
                    nc.scalar.activation(out=dfs, in_=dfp, func=AF.Identity,
                                         bias=bdf_l(l, c))
                    m = work.tile([128, 4 * N], f16, tag=f"m{gi % 2}{c}",
                                  name="m")[:, 0:NG]
                    nc.vector.tensor_mul(m, cfs, dfs)
                    ms.append(m)
                _emit_h(gi, l, ms)

            def _emit_h(gi, l, ms):
                st = gstate[gi]
                NG = st["NG"]
                hpool = psA if gi == 0 else psB
                hp = hpool.tile([F, 4 * N], f32, tag="h", name="hp")[:, 0:NG]
                nc.tensor.matmul(hp, wfc_l(l, 0), ms[0], start=True, stop=False)
                nc.tensor.matmul(hp, wfc_l(l, 1), ms[1], start=False, stop=True)
                hsb = work.tile([F, 4 * N], f16, tag=f"hsb{gi % 2}",
                                name="hsb")[:, 0:NG]
                nc.scalar.activation(out=hsb, in_=hp, func=AF.Copy)
                th = work.tile([F, 4 * N], f16, tag=f"th{gi % 2}",
                               name="th")[:, 0:NG]
                nc.scalar.activation(out=th, in_=hp, func=AF.Tanh)
                st["hsb"], st["th"] = hsb, th

            def emit_head(gi):
                st = gstate[gi]
                NG, bs = st["NG"], st["bs"]
                G = len(bs)
                hd = psS.tile([1, 4 * N], f32, tag="hd", name="hd")[:, 0:NG]
                nc.tensor.matmul(hd, head_w, st["hsb"], start=True, stop=False)
                nc.tensor.matmul(hd, head_w, st["th"], start=False, stop=True)
                nc.vector.tensor_reduce(
                    out=out_acc[0:1, bs[0] : bs[0] + G],
                    in_=hd.rearrange("o (b n) -> o b n", b=G),
                    axis=AX.X,
                    op=mybir.AluOpType.add,
                )

            # ---- tail-group latency path (single-batch groups) ---------
            # The critical input per layer is cf (depends on previous layer's
            # h), so cf accumulates in PSUM via a bias-matmul (ones-row rhs)
            # + mm(hsb) + mm(th); df is dsT-only and is precomputed into
            # SBUF f16 (ACT bias copy) off the critical path.  The DVE mul
            # reads dfs(SBUF) x cfp(PSUM) -- one PSUM operand, 1x mode.
            gl = {}

            def emit_glast_pre(gi):
                """x-only work: layer-0 cf into SBUF f16."""
                bs = GROUPS[gi]
                NG = len(bs) * N
                st = gl.setdefault(gi, {})
                st["NG"], st["bs"] = NG, bs
                cfs0 = []
                for c in range(2):
                    cfp = ps_pool(gi, c).tile([128, 4 * N], f32, tag="cf",
                                              name="cfpL")[:, 0:NG]
                    nc.tensor.matmul(cfp, wcf_l(0, c), xcols(bs[0], bs[-1] + 1),
                                     start=True, stop=True)
                    cs = work.tile([128, 2 * N], f16, tag=f"glcf{c}",
                                   name="glcfs", bufs=1)[:, 0:NG]
                    nc.scalar.activation(out=cs, in_=cfp, func=AF.Identity,
                                         bias=bcf_l(0, c))
                    cfs0.append(cs)
                st["cfs0"] = cfs0

            def emit_glast_trs(gi):
                st = gl[gi]
                bs, NG = st["bs"], st["NG"]
                dsT = dsum_pool.tile([128, 4 * N], f16, tag="dsT",
                                     name=f"dsTL{gi}")
                for k, b in enumerate(bs):
                    trp = psS.tile([128, N], f16, tag="tr", name="trpL")
                    nc.tensor.transpose(trp, dsums.pop(b), ident)
                    nc.scalar.activation(
                        out=dsT[:, k * N : (k + 1) * N], in_=trp, func=AF.Copy
                    )
                st["dsT"] = dsT[:, 0:NG]

            def emit_glast_dfs(gi, l):
                """Precompute df for layer l>=1 into SBUF (off-critical)."""
                st = gl[gi]
                NG = st["NG"]
                res = []
                for c in range(2):
                    dfp = ps_pool(gi, c).tile([128, 4 * N], f32, tag="df",
                                              name="dfpL")[:, 0:NG]
                    nc.tensor.matmul(dfp, wdf_l(l, c), st["dsT"],
                                     start=True, stop=True)
                    ds = work.tile([128, 2 * N], f16, tag=f"gldf{c}{l}",
                                   name="gldfs", bufs=1)[:, 0:NG]
                    nc.scalar.activation(out=ds, in_=dfp, func=AF.Identity,
                                         bias=bdf_l(l, c))
                    res.append(ds)
                st[f"dfs{l}"] = res

            def emit_glast_layer(gi, l):
                st = gl[gi]
                NG = st["NG"]
                ms = []
                for c in range(2):
                    m = work.tile([128, 2 * N], f16, tag=f"glm{c}",
                                  name="glm", bufs=2)[:, 0:NG]
                    if l == 0:
                        # df in PSUM; bias + mul fused on DVE; cf from SBUF
                        dfp = ps_pool(gi, c).tile([128, 4 * N], f32, tag="df",
                                                  name="dfpL")[:, 0:NG]
                        nc.tensor.matmul(dfp, wdf_l(l, c), st["dsT"],
                                         start=True, stop=True)
                        nc.vector.scalar_tensor_tensor(
                            out=m, in0=dfp, scalar=bdf_l(l, c),
                            in1=st["cfs0"][c],
                            op0=mybir.AluOpType.add, op1=mybir.AluOpType.mult)
                    else:
                        # cf in PSUM; bias + mul fused; df from SBUF
                        nc.vector.scalar_tensor_tensor(
                            out=m, in0=st["cfp"][c], scalar=bcf_l(l, c),
                            in1=st[f"dfs{l}"][c],
                            op0=mybir.AluOpType.add, op1=mybir.AluOpType.mult)
                    ms.append(m)
                hp = (psA if l % 2 == 0 else psB).tile(
                    [F, 4 * N], f32, tag="h", name="hpL")[:, 0:NG]
                nc.tensor.matmul(hp, wfc_l(l, 0), ms[0], start=True, stop=False)
                nc.tensor.matmul(hp, wfc_l(l, 1), ms[1], start=False, stop=True)
                th = work.tile([F, 2 * N], f16, tag="glth", name="glth",
                               bufs=2)[:, 0:NG]
                nc.scalar.activation(out=th, in_=hp, func=AF.Tanh)
                xn = work.tile([F, 2 * N], f16, tag="glxn", name="glxn",
                               bufs=2)[:, 0:NG]
                nc.vector.tensor_add(xn, hp, th)
                if l < L - 1:
                    cfps = []
                    for c in range(2):
                        cfp = ps_pool(gi, c).tile([128, 4 * N], f32, tag="cf",
                                                  name="cfpL")[:, 0:NG]
                        nc.tensor.matmul(cfp, wcf_l(l + 1, c), xn,
                                         start=True, stop=True)
                        cfps.append(cfp)
                    st["cfp"] = cfps
                else:
                    st["xn"] = xn

            def emit_glast_head(gi):
                st = gl[gi]
                NG, bs = st["NG"], st["bs"]
                G = len(bs)
                hd = psS.tile([1, 4 * N], f32, tag="hd", name="hdL")[:, 0:NG]
                nc.tensor.matmul(hd, head_w, st["xn"], start=True, stop=True)
                nc.vector.tensor_reduce(
                    out=out_acc[0:1, bs[0] : bs[0] + G],
                    in_=hd.rearrange("o (b n) -> o b n", b=G),
                    axis=AX.X,
                    op=mybir.AluOpType.add,
                )

            # ---- schedule ---------------------------------------------
            fold_full(0, then_dma=((3,),))
            fold_full(1, then_dma=((4,),))
            fold_full(2, then_dma=((5,),))
            emit_glast_pre(2)
            fold_full(3, then_dma=((6,),))
            emit_trs(0)
            emit_layer(0, 0)
            fold_full(4, then_dma=((7,),))
            emit_layer(0, 1)
            fold_full(5)
            emit_layer(0, 2)
            emit_head(0)
            emit_trs(1)
            emit_layer(1, 0)
            fold_full(6)
            emit_layer(1, 1)
            fold_half(7, 0)
            fold_half(7, 1, pop=True)
            emit_layer(1, 2)
            emit_head(1)
            emit_glast_trs(2)
            emit_glast_layer(2, 0)
            emit_glast_dfs(2, 1)
            emit_glast_layer(2, 1)
            emit_glast_dfs(2, 2)
            emit_glast_layer(2, 2)
            emit_glast_head(2)

            nc.sync.dma_start(out=out_ext.rearrange("b o -> o b"), in_=out_acc)

    return nc


def _host_pack(x, Wcf_w, Wcf_b, Wdf_w, Wdf_b, Wfc_w, fc0_w, fc0_b, out_w, out_b):
    f = np.float32
    h = np.float16

    def pack_bf(a):  # (rows, 2K) f16 -> (rows, K) fp32 bit-packed
        return np.ascontiguousarray(a.astype(h)).view(f)

    base = np.zeros((128, WCOLS), f)
    base[:, 0:384] = pack_bf(np.asarray(Wcf_w, f).transpose(2, 0, 1).reshape(128, L * H))
    base[:, 384:768] = pack_bf(
        np.asarray(Wfc_w, f).reshape(L, F, 2, 128).transpose(3, 0, 2, 1).reshape(128, L * 2 * F)
    )
    base[:, BCF_OFF : BCF_OFF + 6] = (
        np.asarray(Wcf_b, f).reshape(L, 2, 128).transpose(2, 0, 1).reshape(128, 6)
    )
    base[:, BDF_OFF : BDF_OFF + 6] = (
        (N * np.asarray(Wdf_b, f)).reshape(L, 2, 128).transpose(2, 0, 1).reshape(128, 6)
    )
    w_head = (np.asarray(out_w, np.float64) @ np.asarray(fc0_w, np.float64))[0]  # (F,)
    head_pair = np.zeros((128, 2), f)
    head_pair[:, 0] = w_head.astype(f)
    base[:, HEAD_OFF : HEAD_OFF + 1] = pack_bf(head_pair)
    # wdf2: rows (j2*64 + r) both halves = Wdf_w[l, h, r]
    wdf2 = np.zeros((128, L * H), f)
    wt = np.asarray(Wdf_w, f).transpose(2, 0, 1).reshape(R, L * H)
    wdf2[0:R] = wt
    wdf2[R:128] = wt
    base[:, WDF_OFF : WDF_OFF + 384] = pack_bf(wdf2)
    base[:, IDOFF : IDOFF + 64] = pack_bf(np.eye(128, dtype=f))
    b_head = float((np.asarray(out_w, np.float64) @ np.asarray(fc0_b, np.float64)
                    + np.asarray(out_b, np.float64)).reshape(()))

    x_t = np.asarray(x, f).transpose(0, 2, 1)  # (B, F, N)
    wpacks = []
    for i in range(NCORES):
        wp = base.copy()
        wp[:, XOFF : XOFF + BL * N // 2] = pack_bf(
            x_t[i * BL : (i + 1) * BL].transpose(1, 0, 2).reshape(128, BL * N)
        )
        wpacks.append(wp)
    return wpacks, b_head


def run(trace=False, **inputs):
    from concourse.bass_utils import run_bass_kernel_spmd

    dist16 = np.ascontiguousarray(
        np.asarray(inputs["distance"]).astype(np.float16).reshape(B, N, N * R)
    )
    wpacks, b_head = _host_pack(
        inputs["x"], inputs["Wcf_w"], inputs["Wcf_b"], inputs["Wdf_w"], inputs["Wdf_b"],
        inputs["Wfc_w"], inputs["fc0_w"], inputs["fc0_b"], inputs["out_w"], inputs["out_b"],
    )

    if "nc" not in _CACHE:
        nc = _build_program()
        nc.finalize()
        _CACHE["nc"] = nc
    nc = _CACHE["nc"]

    in_maps = []
    for i in range(NCORES):
        in_maps.append({
            "dist": np.ascontiguousarray(dist16[i * BL : (i + 1) * BL]),
            "wpack": wpacks[i],
        })
    res = run_bass_kernel_spmd(nc, in_maps, list(range(NCORES)), trace=trace)
    out = np.concatenate([res.results[i]["out"] for i in range(NCORES)], axis=0)
    out = (out.astype(np.float64) + b_head).astype(np.float32)
    return out, res


def kernel(**inputs):
    out, _ = run(trace=False, **inputs)
    return out


# revision 23
# speedup vs baseline: 1.0006x; 1.0006x over previous
"""DeepTensorNeuralNetwork (DTNN / gnn_message_passing) Trainium2 kernel.

Math (per reference):
    d_sum = distance.sum(axis=2)                                  # (B,N,R)
    for l in 0..2:
        cf = x @ Wcf[l].T + bcf[l]                                # (B,N,H)
        df = d_sum @ Wdf[l].T + N*bdf[l]                          # (B,N,H)
        h  = (cf*df) @ Wfc[l].T                                   # (B,N,F)
        x  = h + tanh(h)
    g = x.sum(axis=1); out = (g @ fc0.T + b0) @ ow.T + ob         # (B,1)

Strategy: data-parallel over batch across 8 NeuronCores (8 batches/core).
`distance` is cast to fp16 on the host (like the baseline already did for
x and all weights; quantization error of the j-sum is ~1e-4 relative vs
the 2e-2 gate), which halves the dominant HBM stream to 2.1 MB/batch
(~5.0 us/batch at the measured ~420 GB/s; 17.8 MB total -> ~44 us).
Every large DMA is split across the sync+scalar HWDGE rings to keep the
two rings byte-balanced (an unbalanced ring delays that ring's batches).

The j-reduction is a DVE binary fold tree, all levels fp16 in the 2x
mode (~0.52 ns/output/lane + ~150 ns/op overhead, so FEWER/WIDER ops
win: full 8192->128 trees, 6 ops/batch).  The tree stops at 128 cols =
(j2, r): the last 2:1 j-fold is absorbed into the df matmul by doubling
the Wdf lhsT rows (contract 128 instead of 64; free).  The last batch
uses two half-trees so the final dsum half only gates on the last DMA
chunk.  Measured limits: DVE total work (~48 us: folds 40 + muls 4 +
tail 4) slightly exceeds the stream, so dsum(b7) lands ~10 us after
stream-end; GpSimd tensor ops run ~3.8 ns/elem (no use; they do NOT
serialize against DVE, contrary to the old note, just slow); ACT is the
co-bottleneck (~34 us: 4 bias copies + copy/tanh per layer-group at
NG-proportional cost, exec-queue depth 0).

The 3-layer pipeline runs per batch-group (4,2,2).  For the throughput
groups G0/G1: PE matmuls -> ACT bias copies (fp16 SBUF) -> one DVE mul
per H-half (2x mode), residual handled as cf_{l+1} = mm(Wcf,h_sb) +
mm(Wcf,th) accumulated in PSUM (no DVE adds).  The final group [6,7]
is latency-optimized (it runs after the stream ends): df for l>=1 is
precomputed into SBUF off the critical path, cf accumulates in PSUM,
and the DVE does m = (cf_psum + bias) * df_sbuf in ONE fused
scalar_tensor_tensor; the residual is tanh -> one DVE add, and the
head is a single matmul on x3.  PSUM: bank sets A (cf,df,h) for G0,
B for G1, A+B c-split for the final group, + shared tr/hd = 8 banks.

Known fixed costs: ~6 us preamble (excluded from exec time), ~9.5 us
teardown (288 per-event semaphore resets, framework-fixed).  The core
DVFS state adds run-to-run variance: warm consecutive runs measure
~82-84 us; cold or power-capped runs run all engines at 5/6 clock and
measure ~90-100 us.
"""

import numpy as np

B, N, F, R, H = 64, 128, 128, 64, 256
L = 3
NCORES = 8
BL = B // NCORES   # batches per core
GROUPS = ((0, 1, 2, 3), (4, 5), (6, 7))
GP_SET = ()        # GpSimd fold assist: measured ~3.8 ns/elem (10x slower
                   # than DVE 2x) -- gating any fold level on it loses time

# wpack layout, fp32 columns (bf-cols = fp16-element columns of the
# bitcast view at 2x the fp32 column index):
#   [0, 384)      wcf lhsT f16 : bf-col l*H+h           = Wcf_w[l, h, f]
#   [384, 768)    wfc lhsT f16 : bf-col (l*2+c)*F+f     = Wfc_w[l, f, c*128+hc]
#   [768, 774)    cf bias fp32 : col l*2+c              = Wcf_b[l, c*128+h]
#   [774, 780)    df bias fp32 : col l*2+c              = N * Wdf_b[l, c*128+h]
#   [780, 781)    head lhsT f16: bf-col 0               = (out_w @ fc0_w)[0, f]
#   [784, 1168)   wdf2 lhsT f16: bf-col l*H+h, row j2*64+r = Wdf_w[l, h, r]
#   [1168, 1680)  x f16        : bf-col b*N+n           = x[b_local, n, f]
#   [1680, 1744)  identity f16-packed
BCF_OFF = 768
BDF_OFF = 774
HEAD_OFF = 780
WDF_OFF = 784
XOFF = 1168
IDOFF = 1680
WCOLS = 1744

_CACHE = {}


def _build_program():
    import concourse.bass as bass
    from concourse import bacc
    import concourse.tile as tile
    from concourse import mybir
    from concourse import hw_specs

    # The Tile scheduler orders the (runtime in-order) engine queues from a
    # simulation that models HBM at ~332 GB/s; this device streams ~420.
    # Build with the measured rate so simulated data-arrival (and hence the
    # queue order) matches hardware; restored right after the build.
    _dma_cycle_prev = hw_specs.TRN2Spec.DMA_CYCLE
    hw_specs.TRN2Spec.DMA_CYCLE = 1e9 / (420e9 / 128)
    try:
        return _build_program_inner(bass, bacc, tile, mybir)
    finally:
        hw_specs.TRN2Spec.DMA_CYCLE = _dma_cycle_prev


def _build_program_inner(bass, bacc, tile, mybir):

    f32 = mybir.dt.float32
    f16 = mybir.dt.float16
    AX = mybir.AxisListType
    AF = mybir.ActivationFunctionType

    nc = bacc.Bacc("TRN2")
    dist = nc.declare_dram_parameter("dist", [BL, N, N * R], f16, isOutput=False)
    wpack = nc.declare_dram_parameter("wpack", [128, WCOLS], f32, isOutput=False)
    out_ext = nc.declare_dram_parameter("out", [BL, 1], f32, isOutput=True)

    with tile.TileContext(nc) as tc:
        with (
            tc.tile_pool(name="consts", bufs=1) as consts,
            tc.tile_pool(name="dist", bufs=4) as dist_pool,
            tc.tile_pool(name="fold", bufs=2) as fold_pool,
            tc.tile_pool(name="dsum", bufs=2) as dsum_pool,
            tc.tile_pool(name="work", bufs=2) as work,
            tc.tile_pool(name="psA", bufs=1, space="PSUM") as psA,
            tc.tile_pool(name="psB", bufs=1, space="PSUM") as psB,
            tc.tile_pool(name="psS", bufs=1, space="PSUM") as psS,
        ):
            # ---- DMA: distance stream on sync+scalar rings ------------
            # Stream order interleaves the LAST group's (b6, b7) halves so
            # their first halves fold during early DVE idle and only their
            # second halves arrive at stream end (dsum_b7 ~ stream-bound).
            dist_tiles = {}

            def start_dist_dma(b, hf=None):
                if b in dist_tiles:
                    t = dist_tiles[b]
                else:
                    tag = "dist67" if b >= 6 else "dist"
                    bufs = 2 if b >= 6 else 4
                    t = dist_pool.tile([N, N * R], f16, tag=tag,
                                       name="dist_t", bufs=bufs)
                    dist_tiles[b] = t
                dflat = dist[b, :, :]
                lo = 0 if hf in (None, 0) else 4096
                hi = 8192 if hf in (None, 1) else 4096
                mid = (lo + hi) // 2
                nc.sync.dma_start(out=t[:, lo:mid], in_=dflat[:, lo:mid])
                nc.scalar.dma_start(out=t[:, mid:hi], in_=dflat[:, mid:hi])

            start_dist_dma(0)
            start_dist_dma(1)

            wp = consts.tile([128, WCOLS], f32)
            hwc = WCOLS // 2
            nc.sync.dma_start(out=wp[:, 0:hwc], in_=wpack[:, 0:hwc])
            nc.scalar.dma_start(out=wp[:, hwc:WCOLS],
                                in_=wpack[:, hwc:WCOLS])
            wb = wp.bitcast(f16)  # (128, 2*WCOLS) f16 view
            ident = wb[:, 2 * IDOFF : 2 * IDOFF + 128]
            out_acc = consts.tile([1, BL], f32)

            start_dist_dma(2)

            def wcf_l(l, c):
                o = l * H + c * 128
                return wb[:, o : o + 128]

            def wdf_l(l, c):
                o = 2 * WDF_OFF + l * H + c * 128
                return wb[:, o : o + 128]

            def wfc_l(l, c):
                o = 2 * 384 + (l * 2 + c) * F
                return wb[:, o : o + F]

            def bcf_l(l, c):
                o = BCF_OFF + l * 2 + c
                return wp[:, o : o + 1]

            def bdf_l(l, c):
                o = BDF_OFF + l * 2 + c
                return wp[:, o : o + 1]

            head_w = wb[:, 2 * HEAD_OFF : 2 * HEAD_OFF + 1]

            def xcols(b0, b1):
                return wb[:, 2 * XOFF + b0 * N : 2 * XOFF + b1 * N]

            # ---- folds -------------------------------------------------
            dsums = {}

            def fold_full(b, gp=False, then_dma=()):
                """One tree 8192 -> 128 cols (j2, r)."""
                src = dist_tiles.pop(b)
                for args in then_dma:
                    start_dist_dma(*args)
                s = fold_pool.tile([N, 4096], f16, tag="s", name="s")
                if gp:
                    nc.gpsimd.tensor_add(s[:, 0:2048], src[:, 0:2048],
                                         src[:, 4096:6144])
                    nc.vector.tensor_add(s[:, 2048:4096], src[:, 2048:4096],
                                         src[:, 6144:8192])
                else:
                    nc.vector.tensor_add(s, src[:, 0:4096], src[:, 4096:8192])
                t = fold_pool.tile([N, 2048], f16, tag="t", name="t")
                dsum = dsum_pool.tile([N, 128], f16, tag="dsum", name="dsum", bufs=8)
                cur, other, w = s, t, 2048
                while w >= 128:
                    dst = dsum if w == 128 else other[:, 0:w]
                    nc.vector.tensor_add(dst, cur[:, 0:w], cur[:, w : 2 * w])
                    cur, other = other, cur
                    w //= 2
                dsums[b] = dsum

            def fold_half(b, hf, pop=False):
                """Half-tree (j-range hf) -> dsum[:, hf*64 : hf*64+64]."""
                src_t = dist_tiles[b]
                if b not in dsums:
                    dsums[b] = dsum_pool.tile([N, 128], f16, tag="dsum",
                                              name="dsumh", bufs=8)
                dsum = dsums[b]
                off = hf * 4096
                s = fold_pool.tile([N, 2048], f16, tag=f"hs{hf}", name="hs")
                nc.vector.tensor_add(s, src_t[:, off : off + 2048],
                                     src_t[:, off + 2048 : off + 4096])
                t = fold_pool.tile([N, 1024], f16, tag=f"ht{hf}", name="ht")
                cur, other, w = s, t, 1024
                while w >= 64:
                    dst = dsum[:, hf * 64 : hf * 64 + 64] if w == 64 \
                        else other[:, 0:w]
                    nc.vector.tensor_add(dst, cur[:, 0:w], cur[:, w : 2 * w])
                    cur, other = other, cur
                    w //= 2
                if pop:
                    dist_tiles.pop(b)

            # ---- group state / layer pipeline --------------------------
            gstate = {}

            def ps_pool(gi, c=0):
                """G0 -> A, G1 -> B, final group -> A (c=0) / B (c=1)."""
                if gi == 0:
                    return psA
                if gi == 1:
                    return psB
                return psA if c == 0 else psB

            def emit_trs(gi):
                bs = GROUPS[gi]
                NG = len(bs) * N
                dsT = dsum_pool.tile([128, 4 * N], f16, tag="dsT",
                                     name=f"dsT{gi}")
                for k, b in enumerate(bs):
                    trp = psS.tile([128, N], f16, tag="tr", name="trp")
                    nc.tensor.transpose(trp, dsums.pop(b), ident)
                    nc.scalar.activation(
                        out=dsT[:, k * N : (k + 1) * N], in_=trp, func=AF.Copy
                    )
                gstate[gi] = {"dsT": dsT[:, 0:NG], "NG": NG, "bs": bs,
                              "xc": xcols(bs[0], bs[-1] + 1)}

            def emit_layer(gi, l):
                """Throughput path (G0/G1): ACT bias copies, f16 SBUF muls.
                c halves share one cf bank and one df bank serially."""
                st = gstate[gi]
                NG = st["NG"]
                ms = []
                for c in range(2):
                    cfp = ps_pool(gi, 0).tile([128, 4 * N], f32, tag="cf",
                                              name="cfp")[:, 0:NG]
                    if l == 0:
                        nc.tensor.matmul(cfp, wcf_l(l, c), st["xc"],
                                         start=True, stop=True)
                    else:
                        nc.tensor.matmul(cfp, wcf_l(l, c), st["hsb"],
                                         start=True, stop=False)
                        nc.tensor.matmul(cfp, wcf_l(l, c), st["th"],
                                         start=False, stop=True)
                    cfs = work.tile([128, 4 * N], f16, tag=f"cfs{gi % 2}{c}",
                                    name="cfs")[:, 0:NG]
                    nc.scalar.activation(out=cfs, in_=cfp, func=AF.Identity,
                                         bias=bcf_l(l, c))
                    dfp = ps_pool(gi, 1).tile([128, 4 * N], f32, tag="df",
                                              name="dfp")[:, 0:NG]
                    nc.tensor.matmul(dfp, wdf_l(l, c), st["dsT"],
                                     start=True, stop=True)
                    dfs = work.tile([128, 4 * N], f16, tag=f"dfs{gi % 2}{c}",
                                    name="dfs")[:, 0:NG]
                    nc.scalar.activation(out=dfs, in_=dfp, func=AF.Identity,
                                         bias=bdf_l(l, c))
                    m = work.tile([128, 4 * N], f16, tag=f"m{gi % 2}{c}",
                                  name="m")[:, 0:NG]
                    nc.vector.tensor_mul(m, cfs, dfs)
                    ms.append(m)
                _emit_h(gi, l, ms)

            def _emit_h(gi, l, ms):
                st = gstate[gi]
                NG = st["NG"]
                hpool = psA if gi == 0 else psB
                hp = hpool.tile([F, 4 * N], f32, tag="h", name="hp")[:, 0:NG]
                nc.tensor.matmul(hp, wfc_l(l, 0), ms[0], start=True, stop=False)
                nc.tensor.matmul(hp, wfc_l(l, 1), ms[1], start=False, stop=True)
                hsb = work.tile([F, 4 * N], f16, tag=f"hsb{gi % 2}",
                                name="hsb")[:, 0:NG]
                nc.scalar.activation(out=hsb, in_=hp, func=AF.Copy)
                th = work.tile([F, 4 * N], f16, tag=f"th{gi % 2}",
                               name="th")[:, 0:NG]
                nc.scalar.activation(out=th, in_=hp, func=AF.Tanh)
                st["hsb"], st["th"] = hsb, th

            def emit_head(gi):
                st = gstate[gi]
                NG, bs = st["NG"], st["bs"]
                G = len(bs)
                hd = psS.tile([1, 4 * N], f32, tag="hd", name="hd")[:, 0:NG]
                nc.tensor.matmul(hd, head_w, st["hsb"], start=True, stop=False)
                nc.tensor.matmul(hd, head_w, st["th"], start=False, stop=True)
                nc.vector.tensor_reduce(
                    out=out_acc[0:1, bs[0] : bs[0] + G],
                    in_=hd.rearrange("o (b n) -> o b n", b=G),
                    axis=AX.X,
                    op=mybir.AluOpType.add,
                )

            # ---- tail-group latency path (single-batch groups) ---------
            # The critical input per layer is cf (depends on previous layer's
            # h), so cf accumulates in PSUM via a bias-matmul (ones-row rhs)
            # + mm(hsb) + mm(th); df is dsT-only and is precomputed into
            # SBUF f16 (ACT bias copy) off the critical path.  The DVE mul
            # reads dfs(SBUF) x cfp(PSUM) -- one PSUM operand, 1x mode.
            gl = {}

            def emit_glast_pre(gi):
                """x-only work: layer-0 cf into SBUF f16."""
                bs = GROUPS[gi]
                NG = len(bs) * N
                st = gl.setdefault(gi, {})
                st["NG"], st["bs"] = NG, bs
                cfs0 = []
                for c in range(2):
                    cfp = ps_pool(gi, c).tile([128, 4 * N], f32, tag="cf",
                                              name="cfpL")[:, 0:NG]
                    nc.tensor.matmul(cfp, wcf_l(0, c), xcols(bs[0], bs[-1] + 1),
                                     start=True, stop=True)
                    cs = work.tile([128, 2 * N], f16, tag=f"glcf{c}",
                                   name="glcfs", bufs=1)[:, 0:NG]
                    nc.scalar.activation(out=cs, in_=cfp, func=AF.Identity,
                                         bias=bcf_l(0, c))
                    cfs0.append(cs)
                st["cfs0"] = cfs0

            def emit_glast_trs(gi):
                st = gl[gi]
                bs, NG = st["bs"], st["NG"]
                dsT = dsum_pool.tile([128, 4 * N], f16, tag="dsT",
                                     name=f"dsTL{gi}")
                for k, b in enumerate(bs):
                    trp = psS.tile([128, N], f16, tag="tr", name="trpL")
                    nc.tensor.transpose(trp, dsums.pop(b), ident)
                    nc.scalar.activation(
                        out=dsT[:, k * N : (k + 1) * N], in_=trp, func=AF.Copy
                    )
                st["dsT"] = dsT[:, 0:NG]

            def emit_glast_dfs(gi, l):
                """Precompute df for layer l>=1 into SBUF (off-critical)."""
                st = gl[gi]
                NG = st["NG"]
                res = []
                for c in range(2):
                    dfp = ps_pool(gi, c).tile([128, 4 * N], f32, tag="df",
                                              name="dfpL")[:, 0:NG]
                    nc.tensor.matmul(dfp, wdf_l(l, c), st["dsT"],
                                     start=True, stop=True)
                    ds = work.tile([128, 2 * N], f16, tag=f"gldf{c}{l}",
                                   name="gldfs", bufs=1)[:, 0:NG]
                    nc.scalar.activation(out=ds, in_=dfp, func=AF.Identity,
                                         bias=bdf_l(l, c))
                    res.append(ds)
                st[f"dfs{l}"] = res

            def emit_glast_layer(gi, l):
                st = gl[gi]
                NG = st["NG"]
                ms = []
                for c in range(2):
                    m = work.tile([128, 2 * N], f16, tag=f"glm{c}",
                                  name="glm", bufs=2)[:, 0:NG]
                    if l == 0:
                        # df in PSUM; bias + mul fused on DVE; cf from SBUF
                        dfp = ps_pool(gi, c).tile([128, 4 * N], f32, tag="df",
                                                  name="dfpL")[:, 0:NG]
                        nc.tensor.matmul(dfp, wdf_l(l, c), st["dsT"],
                                         start=True, stop=True)
                        nc.vector.scalar_tensor_tensor(
                            out=m, in0=dfp, scalar=bdf_l(l, c),
                            in1=st["cfs0"][c],
                            op0=mybir.AluOpType.add, op1=mybir.AluOpType.mult)
                    else:
                        # cf in PSUM; bias + mul fused; df from SBUF
                        nc.vector.scalar_tensor_tensor(
                            out=m, in0=st["cfp"][c], scalar=bcf_l(l, c),
                            in1=st[f"dfs{l}"][c],
                            op0=mybir.AluOpType.add, op1=mybir.AluOpType.mult)
                    ms.append(m)
                hp = (psA if l % 2 == 0 else psB).tile(
                    [F, 4 * N], f32, tag="h", name="hpL")[:, 0:NG]
                nc.tensor.matmul(hp, wfc_l(l, 0), ms[0], start=True, stop=False)
                nc.tensor.matmul(hp, wfc_l(l, 1), ms[1], start=False, stop=True)
                th = work.tile([F, 2 * N], f16, tag="glth", name="glth",
                               bufs=2)[:, 0:NG]
                nc.scalar.activation(out=th, in_=hp, func=AF.Tanh)
                xn = work.tile([F, 2 * N], f16, tag="glxn", name="glxn",
                               bufs=2)[:, 0:NG]
                nc.vector.tensor_add(xn, hp, th)
                if l < L - 1:
                    cfps = []
                    for c in range(2):
                        cfp = ps_pool(gi, c).tile([128, 4 * N], f32, tag="cf",
                                                  name="cfpL")[:, 0:NG]
                        nc.tensor.matmul(cfp, wcf_l(l + 1, c), xn,
                                         start=True, stop=True)
                        cfps.append(cfp)
                    st["cfp"] = cfps
                else:
                    st["xn"] = xn

            def emit_glast_head(gi):
                st = gl[gi]
                NG, bs = st["NG"], st["bs"]
                G = len(bs)
                hd = psS.tile([1, 4 * N], f32, tag="hd", name="hdL")[:, 0:NG]
                nc.tensor.matmul(hd, head_w, st["xn"], start=True, stop=True)
                nc.vector.tensor_reduce(
                    out=out_acc[0:1, bs[0] : bs[0] + G],
                    in_=hd.rearrange("o (b n) -> o b n", b=G),
                    axis=AX.X,
                    op=mybir.AluOpType.add,
                )

            # ---- schedule ---------------------------------------------
            fold_full(0, then_dma=((3,),))
            fold_full(1, then_dma=((4,),))
            fold_full(2, then_dma=((5,),))
            emit_glast_pre(2)
            fold_full(3, then_dma=((6,),))
            emit_trs(0)
            emit_layer(0, 0)
            fold_full(4, then_dma=((7,),))
            emit_layer(0, 1)
            fold_full(5)
            emit_layer(0, 2)
            emit_head(0)
            emit_trs(1)
            emit_layer(1, 0)
            fold_full(6)
            emit_layer(1, 1)
            fold_half(7, 0)
            fold_half(7, 1, pop=True)
            emit_layer(1, 2)
            emit_head(1)
            emit_glast_trs(2)
            emit_glast_layer(2, 0)
            emit_glast_dfs(2, 1)
            emit_glast_layer(2, 1)
            emit_glast_dfs(2, 2)
            emit_glast_layer(2, 2)
            emit_glast_head(2)

            nc.sync.dma_start(out=out_ext.rearrange("b o -> o b"), in_=out_acc)

    return nc


def _host_pack(x, Wcf_w, Wcf_b, Wdf_w, Wdf_b, Wfc_w, fc0_w, fc0_b, out_w, out_b):
    f = np.float32
    h = np.float16

    def pack_bf(a):  # (rows, 2K) f16 -> (rows, K) fp32 bit-packed
        return np.ascontiguousarray(a.astype(h)).view(f)

    base = np.zeros((128, WCOLS), f)
    base[:, 0:384] = pack_bf(np.asarray(Wcf_w, f).transpose(2, 0, 1).reshape(128, L * H))
    base[:, 384:768] = pack_bf(
        np.asarray(Wfc_w, f).reshape(L, F, 2, 128).transpose(3, 0, 2, 1).reshape(128, L * 2 * F)
    )
    base[:, BCF_OFF : BCF_OFF + 6] = (
        np.asarray(Wcf_b, f).reshape(L, 2, 128).transpose(2, 0, 1).reshape(128, 6)
    )
    base[:, BDF_OFF : BDF_OFF + 6] = (
        (N * np.asarray(Wdf_b, f)).reshape(L, 2, 128).transpose(2, 0, 1).reshape(128, 6)
    )
    w_head = (np.asarray(out_w, np.float64) @ np.asarray(fc0_w, np.float64))[0]  # (F,)
    head_pair = np.zeros((128, 2), f)
    head_pair[:, 0] = w_head.astype(f)
    base[:, HEAD_OFF : HEAD_OFF + 1] = pack_bf(head_pair)
    # wdf2: rows (j2*64 + r) both halves = Wdf_w[l, h, r]
    wdf2 = np.zeros((128, L * H), f)
    wt = np.asarray(Wdf_w, f).transpose(2, 0, 1).reshape(R, L * H)
    wdf2[0:R] = wt
    wdf2[R:128] = wt
    base[:, WDF_OFF : WDF_OFF + 384] = pack_bf(wdf2)
    base[:, IDOFF : IDOFF + 64] = pack_bf(np.eye(128, dtype=f))
    b_head = float((np.asarray(out_w, np.float64) @ np.asarray(fc0_b, np.float64)
                    + np.asarray(out_b, np.float64)).reshape(()))

    x_t = np.asarray(x, f).transpose(0, 2, 1)  # (B, F, N)
    wpacks = []
    for i in range(NCORES):
        wp = base.copy()
        wp[:, XOFF : XOFF + BL * N // 2] = pack_bf(
            x_t[i * BL : (i + 1) * BL].transpose(1, 0, 2).reshape(128, BL * N)
        )
        wpacks.append(wp)
    return wpacks, b_head


def run(trace=False, **inputs):
    from concourse.bass_utils import run_bass_kernel_spmd

    dist16 = np.ascontiguousarray(
        np.asarray(inputs["distance"]).astype(np.float16).reshape(B, N, N * R)
    )
    wpacks, b_head = _host_pack(
        inputs["x"], inputs["Wcf_w"], inputs["Wcf_b"], inputs["Wdf_w"], inputs["Wdf_b"],
        inputs["Wfc_w"], inputs["fc0_w"], inputs["fc0_b"], inputs["out_w"], inputs["out_b"],
    )

    if "nc" not in _CACHE:
        nc = _build_program()
        nc.finalize()
        _CACHE["nc"] = nc
    nc = _CACHE["nc"]

    in_maps = []
    for i in range(NCORES):
        in_maps.append({
            "dist": np.ascontiguousarray(dist16[i * BL : (i + 1) * BL]),
            "wpack": wpacks[i],
        })
    res = run_bass_kernel_spmd(nc, in_maps, list(range(NCORES)), trace=trace)
    out = np.concatenate([res.results[i]["out"] for i in range(NCORES)], axis=0)
    out = (out.astype(np.float64) + b_head).astype(np.float32)
    return out, res


def kernel(**inputs):
    out, _ = run(trace=False, **inputs)
    return out


# revision 25
# speedup vs baseline: 1.0066x; 1.0060x over previous
"""DeepTensorNeuralNetwork (DTNN / gnn_message_passing) Trainium2 kernel.

Math (per reference):
    d_sum = distance.sum(axis=2)                                  # (B,N,R)
    for l in 0..2:
        cf = x @ Wcf[l].T + bcf[l]                                # (B,N,H)
        df = d_sum @ Wdf[l].T + N*bdf[l]                          # (B,N,H)
        h  = (cf*df) @ Wfc[l].T                                   # (B,N,F)
        x  = h + tanh(h)
    g = x.sum(axis=1); out = (g @ fc0.T + b0) @ ow.T + ob         # (B,1)

Strategy: data-parallel over batch across 8 NeuronCores (8 batches/core).
`distance` is cast to fp16 on the host (like the baseline already did for
x and all weights; quantization error of the j-sum is ~1e-4 relative vs
the 2e-2 gate), which halves the dominant HBM stream to 2.1 MB/batch
(~5.0 us/batch at the measured ~420 GB/s; 17.8 MB total -> ~44 us).
Every large DMA is split across the sync+scalar HWDGE rings to keep the
two rings byte-balanced (an unbalanced ring delays that ring's batches).

The j-reduction is a DVE binary fold tree, all levels fp16 in the 2x
mode (~0.52 ns/output/lane + ~150 ns/op overhead, so FEWER/WIDER ops
win: full 8192->128 trees, 6 ops/batch).  The tree stops at 128 cols =
(j2, r): the last 2:1 j-fold is absorbed into the df matmul by doubling
the Wdf lhsT rows (contract 128 instead of 64; free).  The last batch
uses two half-trees so the final dsum half only gates on the last DMA
chunk.  Measured limits: DVE total work (~48 us: folds 40 + muls 4 +
tail 4) slightly exceeds the stream, so dsum(b7) lands ~10 us after
stream-end; GpSimd tensor ops run ~3.8 ns/elem (no use; they do NOT
serialize against DVE, contrary to the old note, just slow); ACT is the
co-bottleneck (~34 us: 4 bias copies + copy/tanh per layer-group at
NG-proportional cost, exec-queue depth 0).

The 3-layer pipeline runs per batch-group (4,2,2).  For the throughput
groups G0/G1: PE matmuls -> ACT bias copies (fp16 SBUF) -> one DVE mul
per H-half (2x mode), residual handled as cf_{l+1} = mm(Wcf,h_sb) +
mm(Wcf,th) accumulated in PSUM (no DVE adds).  The final group [6,7]
is latency-optimized (it runs after the stream ends): df for l>=1 is
precomputed into SBUF off the critical path, cf accumulates in PSUM,
and the DVE does m = (cf_psum + bias) * df_sbuf in ONE fused
scalar_tensor_tensor; the residual is tanh -> one DVE add, and the
head is a single matmul on x3.  PSUM: bank sets A (cf,df,h) for G0,
B for G1, A+B c-split for the final group, + shared tr/hd = 8 banks.

Known fixed costs: ~6 us preamble (excluded from exec time), ~9.5 us
teardown (288 per-event semaphore resets, framework-fixed).  The core
DVFS state adds run-to-run variance: warm consecutive runs measure
~82-84 us; cold or power-capped runs run all engines at 5/6 clock and
measure ~90-100 us.
"""

import numpy as np

B, N, F, R, H = 64, 128, 128, 64, 256
L = 3
NCORES = 8
BL = B // NCORES   # batches per core
GROUPS = ((0, 1, 2, 3), (4, 5), (6, 7))
GP_SET = ()        # GpSimd fold assist: measured ~3.8 ns/elem (10x slower
                   # than DVE 2x) -- gating any fold level on it loses time

# wpack layout, fp32 columns (bf-cols = fp16-element columns of the
# bitcast view at 2x the fp32 column index):
#   [0, 384)      wcf lhsT f16 : bf-col l*H+h           = Wcf_w[l, h, f]
#   [384, 768)    wfc lhsT f16 : bf-col (l*2+c)*F+f     = Wfc_w[l, f, c*128+hc]
#   [768, 774)    cf bias fp32 : col l*2+c              = Wcf_b[l, c*128+h]
#   [774, 780)    df bias fp32 : col l*2+c              = N * Wdf_b[l, c*128+h]
#   [780, 781)    head lhsT f16: bf-col 0               = (out_w @ fc0_w)[0, f]
#   [784, 1168)   wdf2 lhsT f16: bf-col l*H+h, row j2*64+r = Wdf_w[l, h, r]
#   [1168, 1680)  x f16        : bf-col b*N+n           = x[b_local, n, f]
#   [1680, 1744)  identity f16-packed
BCF_OFF = 768
BDF_OFF = 774
HEAD_OFF = 780
WDF_OFF = 784
XOFF = 1168
IDOFF = 1680
WCOLS = 1744

_CACHE = {}


def _build_program():
    import concourse.bass as bass
    from concourse import bacc
    import concourse.tile as tile
    from concourse import mybir
    from concourse import hw_specs

    # The Tile scheduler orders the (runtime in-order) engine queues from a
    # simulation that models HBM at ~332 GB/s; this device streams ~420.
    # Build with the measured rate so simulated data-arrival (and hence the
    # queue order) matches hardware; restored right after the build.
    _dma_cycle_prev = hw_specs.TRN2Spec.DMA_CYCLE
    hw_specs.TRN2Spec.DMA_CYCLE = 1e9 / (420e9 / 128)
    try:
        return _build_program_inner(bass, bacc, tile, mybir)
    finally:
        hw_specs.TRN2Spec.DMA_CYCLE = _dma_cycle_prev


def _build_program_inner(bass, bacc, tile, mybir):

    f32 = mybir.dt.float32
    f16 = mybir.dt.float16
    AX = mybir.AxisListType
    AF = mybir.ActivationFunctionType

    nc = bacc.Bacc("TRN2")
    dist = nc.declare_dram_parameter("dist", [BL, N, N * R], f16, isOutput=False)
    wpack = nc.declare_dram_parameter("wpack", [128, WCOLS], f32, isOutput=False)
    out_ext = nc.declare_dram_parameter("out", [BL, 1], f32, isOutput=True)

    with tile.TileContext(nc) as tc:
        with (
            tc.tile_pool(name="consts", bufs=1) as consts,
            tc.tile_pool(name="dist", bufs=4) as dist_pool,
            tc.tile_pool(name="fold", bufs=2) as fold_pool,
            tc.tile_pool(name="dsum", bufs=2) as dsum_pool,
            tc.tile_pool(name="work", bufs=2) as work,
            tc.tile_pool(name="psA", bufs=1, space="PSUM") as psA,
            tc.tile_pool(name="psB", bufs=1, space="PSUM") as psB,
            tc.tile_pool(name="psS", bufs=1, space="PSUM") as psS,
        ):
            # ---- DMA: distance stream on sync+scalar rings ------------
            # Stream order interleaves the LAST group's (b6, b7) halves so
            # their first halves fold during early DVE idle and only their
            # second halves arrive at stream end (dsum_b7 ~ stream-bound).
            dist_tiles = {}

            def start_dist_dma(b, hf=None):
                if b in dist_tiles:
                    t = dist_tiles[b]
                else:
                    tag = "dist67" if b >= 6 else "dist"
                    bufs = 2 if b >= 6 else 4
                    t = dist_pool.tile([N, N * R], f16, tag=tag,
                                       name="dist_t", bufs=bufs)
                    dist_tiles[b] = t
                dflat = dist[b, :, :]
                lo = 0 if hf in (None, 0) else 4096
                hi = 8192 if hf in (None, 1) else 4096
                mid = (lo + hi) // 2
                nc.sync.dma_start(out=t[:, lo:mid], in_=dflat[:, lo:mid])
                nc.scalar.dma_start(out=t[:, mid:hi], in_=dflat[:, mid:hi])

            start_dist_dma(0)
            start_dist_dma(1)

            wp = consts.tile([128, WCOLS], f32)
            hwc = WCOLS // 2
            nc.sync.dma_start(out=wp[:, 0:hwc], in_=wpack[:, 0:hwc])
            nc.scalar.dma_start(out=wp[:, hwc:WCOLS],
                                in_=wpack[:, hwc:WCOLS])
            wb = wp.bitcast(f16)  # (128, 2*WCOLS) f16 view
            ident = wb[:, 2 * IDOFF : 2 * IDOFF + 128]
            out_acc = consts.tile([1, BL], f32)

            start_dist_dma(2)

            def wcf_l(l, c):
                o = l * H + c * 128
                return wb[:, o : o + 128]

            def wdf_l(l, c):
                o = 2 * WDF_OFF + l * H + c * 128
                return wb[:, o : o + 128]

            def wfc_l(l, c):
                o = 2 * 384 + (l * 2 + c) * F
                return wb[:, o : o + F]

            def bcf_l(l, c):
                o = BCF_OFF + l * 2 + c
                return wp[:, o : o + 1]

            def bdf_l(l, c):
                o = BDF_OFF + l * 2 + c
                return wp[:, o : o + 1]

            head_w = wb[:, 2 * HEAD_OFF : 2 * HEAD_OFF + 1]

            def xcols(b0, b1):
                return wb[:, 2 * XOFF + b0 * N : 2 * XOFF + b1 * N]

            # ---- folds -------------------------------------------------
            dsums = {}

            def fold_full(b, gp=False, then_dma=()):
                """One tree 8192 -> 128 cols (j2, r)."""
                src = dist_tiles.pop(b)
                for args in then_dma:
                    start_dist_dma(*args)
                s = fold_pool.tile([N, 4096], f16, tag="s", name="s")
                if gp:
                    nc.gpsimd.tensor_add(s[:, 0:2048], src[:, 0:2048],
                                         src[:, 4096:6144])
                    nc.vector.tensor_add(s[:, 2048:4096], src[:, 2048:4096],
                                         src[:, 6144:8192])
                else:
                    nc.vector.tensor_add(s, src[:, 0:4096], src[:, 4096:8192])
                t = fold_pool.tile([N, 2048], f16, tag="t", name="t")
                dsum = dsum_pool.tile([N, 128], f16, tag="dsum", name="dsum", bufs=8)
                cur, other, w = s, t, 2048
                while w >= 128:
                    dst = dsum if w == 128 else other[:, 0:w]
                    nc.vector.tensor_add(dst, cur[:, 0:w], cur[:, w : 2 * w])
                    cur, other = other, cur
                    w //= 2
                dsums[b] = dsum

            def fold_half(b, hf, pop=False):
                """Half-tree (j-range hf) -> dsum[:, hf*64 : hf*64+64]."""
                src_t = dist_tiles[b]
                if b not in dsums:
                    dsums[b] = dsum_pool.tile([N, 128], f16, tag="dsum",
                                              name="dsumh", bufs=8)
                dsum = dsums[b]
                off = hf * 4096
                s = fold_pool.tile([N, 2048], f16, tag=f"hs{hf}", name="hs")
                nc.vector.tensor_add(s, src_t[:, off : off + 2048],
                                     src_t[:, off + 2048 : off + 4096])
                t = fold_pool.tile([N, 1024], f16, tag=f"ht{hf}", name="ht")
                cur, other, w = s, t, 1024
                while w >= 64:
                    dst = dsum[:, hf * 64 : hf * 64 + 64] if w == 64 \
                        else other[:, 0:w]
                    nc.vector.tensor_add(dst, cur[:, 0:w], cur[:, w : 2 * w])
                    cur, other = other, cur
                    w //= 2
                if pop:
                    dist_tiles.pop(b)

            # ---- group state / layer pipeline --------------------------
            gstate = {}

            def ps_pool(gi, c=0):
                """G0 -> A, G1 -> B, final group -> A (c=0) / B (c=1)."""
                if gi == 0:
                    return psA
                if gi == 1:
                    return psB
                return psA if c == 0 else psB

            def emit_trs(gi):
                bs = GROUPS[gi]
                NG = len(bs) * N
                dsT = dsum_pool.tile([128, 4 * N], f16, tag="dsT",
                                     name=f"dsT{gi}")
                for k, b in enumerate(bs):
                    trp = psS.tile([128, N], f16, tag="tr", name="trp")
                    nc.tensor.transpose(trp, dsums.pop(b), ident)
                    nc.scalar.activation(
                        out=dsT[:, k * N : (k + 1) * N], in_=trp, func=AF.Copy
                    )
                gstate[gi] = {"dsT": dsT[:, 0:NG], "NG": NG, "bs": bs,
                              "xc": xcols(bs[0], bs[-1] + 1)}

            def emit_layer(gi, l):
                """Throughput path (G0/G1): ACT bias copies, f16 SBUF muls.
                c halves share one cf bank and one df bank serially."""
                st = gstate[gi]
                NG = st["NG"]
                ms = []
                for c in range(2):
                    cfp = ps_pool(gi, 0).tile([128, 4 * N], f32, tag="cf",
                                              name="cfp")[:, 0:NG]
                    if l == 0:
                        nc.tensor.matmul(cfp, wcf_l(l, c), st["xc"],
                                         start=True, stop=True)
                    else:
                        nc.tensor.matmul(cfp, wcf_l(l, c), st["hsb"],
                                         start=True, stop=False)
                        nc.tensor.matmul(cfp, wcf_l(l, c), st["th"],
                                         start=False, stop=True)
                    cfs = work.tile([128, 4 * N], f16, tag=f"cfs{gi % 2}{c}",
                                    name="cfs")[:, 0:NG]
                    nc.scalar.activation(out=cfs, in_=cfp, func=AF.Identity,
                                         bias=bcf_l(l, c))
                    dfp = ps_pool(gi, 1).tile([128, 4 * N], f32, tag="df",
                                              name="dfp")[:, 0:NG]
                    nc.tensor.matmul(dfp, wdf_l(l, c), st["dsT"],
                                     start=True, stop=True)
                    dfs = work.tile([128, 4 * N], f16, tag=f"dfs{gi % 2}{c}",
                                    name="dfs")[:, 0:NG]
                    nc.scalar.activation(out=dfs, in_=dfp, func=AF.Identity,
                                         bias=bdf_l(l, c))
                    m = work.tile([128, 4 * N], f16, tag=f"m{gi % 2}{c}",
                                  name="m")[:, 0:NG]
                    nc.vector.tensor_mul(m, cfs, dfs)
                    ms.append(m)
                _emit_h(gi, l, ms)

            def _emit_h(gi, l, ms):
                st = gstate[gi]
                NG = st["NG"]
                hpool = psA if gi == 0 else psB
                hp = hpool.tile([F, 4 * N], f32, tag="h", name="hp")[:, 0:NG]
                nc.tensor.matmul(hp, wfc_l(l, 0), ms[0], start=True, stop=False)
                nc.tensor.matmul(hp, wfc_l(l, 1), ms[1], start=False, stop=True)
                hsb = work.tile([F, 4 * N], f16, tag=f"hsb{gi % 2}",
                                name="hsb")[:, 0:NG]
                nc.scalar.activation(out=hsb, in_=hp, func=AF.Copy)
                th = work.tile([F, 4 * N], f16, tag=f"th{gi % 2}",
                               name="th")[:, 0:NG]
                nc.scalar.activation(out=th, in_=hp, func=AF.Tanh)
                st["hsb"], st["th"] = hsb, th

            def emit_head(gi):
                st = gstate[gi]
                NG, bs = st["NG"], st["bs"]
                G = len(bs)
                hd = psS.tile([1, 4 * N], f32, tag="hd", name="hd")[:, 0:NG]
                nc.tensor.matmul(hd, head_w, st["hsb"], start=True, stop=False)
                nc.tensor.matmul(hd, head_w, st["th"], start=False, stop=True)
                nc.vector.tensor_reduce(
                    out=out_acc[0:1, bs[0] : bs[0] + G],
                    in_=hd.rearrange("o (b n) -> o b n", b=G),
                    axis=AX.X,
                    op=mybir.AluOpType.add,
                )

            # ---- tail-group latency path (single-batch groups) ---------
            # The critical input per layer is cf (depends on previous layer's
            # h), so cf accumulates in PSUM via a bias-matmul (ones-row rhs)
            # + mm(hsb) + mm(th); df is dsT-only and is precomputed into
            # SBUF f16 (ACT bias copy) off the critical path.  The DVE mul
            # reads dfs(SBUF) x cfp(PSUM) -- one PSUM operand, 1x mode.
            gl = {}

            def emit_glast_pre(gi):
                """x-only work: layer-0 cf into SBUF f16."""
                bs = GROUPS[gi]
                NG = len(bs) * N
                st = gl.setdefault(gi, {})
                st["NG"], st["bs"] = NG, bs
                cfs0 = []
                for c in range(2):
                    cfp = ps_pool(gi, c).tile([128, 4 * N], f32, tag="cf",
                                              name="cfpL")[:, 0:NG]
                    nc.tensor.matmul(cfp, wcf_l(0, c), xcols(bs[0], bs[-1] + 1),
                                     start=True, stop=True)
                    cs = work.tile([128, 2 * N], f16, tag=f"glcf{c}",
                                   name="glcfs", bufs=1)[:, 0:NG]
                    nc.scalar.activation(out=cs, in_=cfp, func=AF.Identity,
                                         bias=bcf_l(0, c))
                    cfs0.append(cs)
                st["cfs0"] = cfs0

            def emit_glast_trs(gi):
                st = gl[gi]
                bs, NG = st["bs"], st["NG"]
                dsT = dsum_pool.tile([128, 4 * N], f16, tag="dsT",
                                     name=f"dsTL{gi}")
                for k, b in enumerate(bs):
                    trp = psS.tile([128, N], f16, tag="tr", name="trpL")
                    nc.tensor.transpose(trp, dsums.pop(b), ident)
                    nc.scalar.activation(
                        out=dsT[:, k * N : (k + 1) * N], in_=trp, func=AF.Copy
                    )
                st["dsT"] = dsT[:, 0:NG]

            def emit_glast_dfs(gi, l):
                """Precompute df for layer l>=1 into SBUF (off-critical)."""
                st = gl[gi]
                NG = st["NG"]
                res = []
                for c in range(2):
                    dfp = ps_pool(gi, c).tile([128, 4 * N], f32, tag="df",
                                              name="dfpL")[:, 0:NG]
                    nc.tensor.matmul(dfp, wdf_l(l, c), st["dsT"],
                                     start=True, stop=True)
                    ds = work.tile([128, 2 * N], f16, tag=f"gldf{c}{l}",
                                   name="gldfs", bufs=1)[:, 0:NG]
                    nc.scalar.activation(out=ds, in_=dfp, func=AF.Identity,
                                         bias=bdf_l(l, c))
                    res.append(ds)
                st[f"dfs{l}"] = res

            def emit_glast_layer(gi, l):
                st = gl[gi]
                NG = st["NG"]
                ms = []
                for c in range(2):
                    m = work.tile([128, 2 * N], f16, tag=f"glm{c}",
                                  name="glm", bufs=2)[:, 0:NG]
                    if l == 0:
                        # df in PSUM; bias + mul fused on DVE; cf from SBUF
                        dfp = ps_pool(gi, c).tile([128, 4 * N], f32, tag="df",
                                                  name="dfpL")[:, 0:NG]
                        nc.tensor.matmul(dfp, wdf_l(l, c), st["dsT"],
                                         start=True, stop=True)
                        nc.vector.scalar_tensor_tensor(
                            out=m, in0=dfp, scalar=bdf_l(l, c),
                            in1=st["cfs0"][c],
                            op0=mybir.AluOpType.add, op1=mybir.AluOpType.mult)
                    else:
                        # cf in PSUM; bias + mul fused; df from SBUF
                        nc.vector.scalar_tensor_tensor(
                            out=m, in0=st["cfp"][c], scalar=bcf_l(l, c),
                            in1=st[f"dfs{l}"][c],
                            op0=mybir.AluOpType.add, op1=mybir.AluOpType.mult)
                    ms.append(m)
                hp = (psA if l % 2 == 0 else psB).tile(
                    [F, 4 * N], f32, tag="h", name="hpL")[:, 0:NG]
                nc.tensor.matmul(hp, wfc_l(l, 0), ms[0], start=True, stop=False)
                nc.tensor.matmul(hp, wfc_l(l, 1), ms[1], start=False, stop=True)
                th = work.tile([F, 2 * N], f16, tag="glth", name="glth",
                               bufs=2)[:, 0:NG]
                nc.scalar.activation(out=th, in_=hp, func=AF.Tanh)
                xn = work.tile([F, 2 * N], f16, tag="glxn", name="glxn",
                               bufs=2)[:, 0:NG]
                nc.vector.tensor_add(xn, hp, th)
                if l < L - 1:
                    cfps = []
                    for c in range(2):
                        cfp = ps_pool(gi, c).tile([128, 4 * N], f32, tag="cf",
                                                  name="cfpL")[:, 0:NG]
                        nc.tensor.matmul(cfp, wcf_l(l + 1, c), xn,
                                         start=True, stop=True)
                        cfps.append(cfp)
                    st["cfp"] = cfps
                else:
                    st["xn"] = xn

            def emit_glast_head(gi):
                st = gl[gi]
                NG, bs = st["NG"], st["bs"]
                G = len(bs)
                hd = psS.tile([1, 4 * N], f32, tag="hd", name="hdL")[:, 0:NG]
                nc.tensor.matmul(hd, head_w, st["xn"], start=True, stop=True)
                nc.vector.tensor_reduce(
                    out=out_acc[0:1, bs[0] : bs[0] + G],
                    in_=hd.rearrange("o (b n) -> o b n", b=G),
                    axis=AX.X,
                    op=mybir.AluOpType.add,
                )

            # ---- schedule ---------------------------------------------
            fold_full(0, then_dma=((3,),))
            fold_full(1, then_dma=((4,),))
            fold_full(2, then_dma=((5,),))
            emit_glast_pre(2)
            fold_full(3, then_dma=((6,),))
            emit_trs(0)
            emit_layer(0, 0)
            fold_full(4, then_dma=((7,),))
            emit_layer(0, 1)
            fold_full(5)
            emit_layer(0, 2)
            emit_head(0)
            emit_trs(1)
            emit_layer(1, 0)
            fold_full(6)
            emit_layer(1, 1)
            fold_half(7, 0)
            fold_half(7, 1, pop=True)
            emit_layer(1, 2)
            emit_head(1)
            emit_glast_trs(2)
            emit_glast_layer(2, 0)
            emit_glast_dfs(2, 1)
            emit_glast_layer(2, 1)
            emit_glast_dfs(2, 2)
            emit_glast_layer(2, 2)
            emit_glast_head(2)

            nc.sync.dma_start(out=out_ext.rearrange("b o -> o b"), in_=out_acc)

    return nc


def _host_pack(x, Wcf_w, Wcf_b, Wdf_w, Wdf_b, Wfc_w, fc0_w, fc0_b, out_w, out_b):
    f = np.float32
    h = np.float16

    def pack_bf(a):  # (rows, 2K) f16 -> (rows, K) fp32 bit-packed
        return np.ascontiguousarray(a.astype(h)).view(f)

    base = np.zeros((128, WCOLS), f)
    base[:, 0:384] = pack_bf(np.asarray(Wcf_w, f).transpose(2, 0, 1).reshape(128, L * H))
    base[:, 384:768] = pack_bf(
        np.asarray(Wfc_w, f).reshape(L, F, 2, 128).transpose(3, 0, 2, 1).reshape(128, L * 2 * F)
    )
    base[:, BCF_OFF : BCF_OFF + 6] = (
        np.asarray(Wcf_b, f).reshape(L, 2, 128).transpose(2, 0, 1).reshape(128, 6)
    )
    base[:, BDF_OFF : BDF_OFF + 6] = (
        (N * np.asarray(Wdf_b, f)).reshape(L, 2, 128).transpose(2, 0, 1).reshape(128, 6)
    )
    w_head = (np.asarray(out_w, np.float64) @ np.asarray(fc0_w, np.float64))[0]  # (F,)
    head_pair = np.zeros((128, 2), f)
    head_pair[:, 0] = w_head.astype(f)
    base[:, HEAD_OFF : HEAD_OFF + 1] = pack_bf(head_pair)
    # wdf2: rows (j2*64 + r) both halves = Wdf_w[l, h, r]
    wdf2 = np.zeros((128, L * H), f)
    wt = np.asarray(Wdf_w, f).transpose(2, 0, 1).reshape(R, L * H)
    wdf2[0:R] = wt
    wdf2[R:128] = wt
    base[:, WDF_OFF : WDF_OFF + 384] = pack_bf(wdf2)
    base[:, IDOFF : IDOFF + 64] = pack_bf(np.eye(128, dtype=f))
    b_head = float((np.asarray(out_w, np.float64) @ np.asarray(fc0_b, np.float64)
                    + np.asarray(out_b, np.float64)).reshape(()))

    x_t = np.asarray(x, f).transpose(0, 2, 1)  # (B, F, N)
    wpacks = []
    for i in range(NCORES):
        wp = base.copy()
        wp[:, XOFF : XOFF + BL * N // 2] = pack_bf(
            x_t[i * BL : (i + 1) * BL].transpose(1, 0, 2).reshape(128, BL * N)
        )
        wpacks.append(wp)
    return wpacks, b_head


def run(trace=False, **inputs):
    from concourse.bass_utils import run_bass_kernel_spmd

    dist16 = np.ascontiguousarray(
        np.asarray(inputs["distance"]).astype(np.float16).reshape(B, N, N * R)
    )
    wpacks, b_head = _host_pack(
        inputs["x"], inputs["Wcf_w"], inputs["Wcf_b"], inputs["Wdf_w"], inputs["Wdf_b"],
        inputs["Wfc_w"], inputs["fc0_w"], inputs["fc0_b"], inputs["out_w"], inputs["out_b"],
    )

    if "nc" not in _CACHE:
        nc = _build_program()
        nc.finalize()
        _CACHE["nc"] = nc
    nc = _CACHE["nc"]

    in_maps = []
    for i in range(NCORES):
        in_maps.append({
            "dist": np.ascontiguousarray(dist16[i * BL : (i + 1) * BL]),
            "wpack": wpacks[i],
        })
    res = run_bass_kernel_spmd(nc, in_maps, list(range(NCORES)), trace=trace)
    out = np.concatenate([res.results[i]["out"] for i in range(NCORES)], axis=0)
    out = (out.astype(np.float64) + b_head).astype(np.float32)
    return out, res


def kernel(**inputs):
    out, _ = run(trace=False, **inputs)
    return out
